# revision 1
# baseline (speedup 1.0000x reference)
"""Causal multi-head attention (fused QKV) on 8 Trainium2 NeuronCores.

Problem: x[2, 2048, 1024] @ W_qkv[1024, 3072] -> causal MHA, 16 heads,
head_dim 64 -> out [2, 2048, 1024].

Sharding: batch (2) x head-groups (4) = 8 shards; core c handles batch
c//4, heads 4*(c%4) .. 4*(c%4)+3.  Each core is fully independent (no
collectives).

v3 design (vs baseline, 184us -> ~141us):
  - all matmul operands bf16 (host converts): halves input DMA, same PE
    rate as fp32r, no FP32-HIGH 4-pass projection.
  - host packs x/w into SBUF-layout-contiguous buffers -> few large DMAs
    with 4-8KB rows instead of many 512B-packet strided ones; weights on
    the gpsimd DMA queue, x on the sync queue (parallel ramp).
  - PE warmup matmuls + ACT table preload run during the DMA head so HAM
    un-throttles (1.2 -> 2.4 GHz) before real work and stays warm.
  - per-chunk software pipeline with projection units emitted as
    FILLERS inside the attention kb-loops (emit_attn(fillers=...)): the
    Tile scheduler places them into ScalarE-exp-bound stretches, so
    neither the PE nor ScalarE ever starves for a whole window (-10us
    vs emitting each window's projection as a block). pr1 attention is
    staggered 2 chunks behind pr0.
  - merged av accumulator [65, 1024] per (pr, qc) with the softmax
    denominator riding in row 0 (ones column first in vcat); normalize
    chain (copy -> partition_broadcast -> reciprocal -> multiply) emitted
    at high priority so it overlaps the next chunk instead of piling up
    at the kernel tail.

Measured dead ends (kept disabled in the code): fp8 P/V with DoubleRow
(rel err 4.5e-2 > 2e-2 gate), custom 2-pass DVE exp offload (lengthens
the per-kb critical chain; net +2-8us), N=1024 moving matmuls (ISA cap
is 512), mask multiply on GpSimd (stalls behind partition_broadcasts).

Per-core layouts (host prepares, all bf16 except biases/output):
  xp   [128, 16384]  x[b].T packed [p, sc, dc, j] (sc=512-chunk, dc=128-deep)
  wqkA [128, 2048]   per dc: [Q01(128) | K01(128)] columns
  wqkB [128, 2048]   per dc: [Q23(128) | K23(128)]
  wv   [128, 2048]   per dc: [V(256)]
  qkb  [128, 4] f32  QK bias per fc; vb [128, 256] f32 V bias
  outT [256, S] f32  row 64*h+j, col s = out[b,s,h,j]
"""

import sys

if "/opt/trn_rl_repo" not in sys.path:
    sys.path.insert(0, "/opt/trn_rl_repo")

import numpy as np
import ml_dtypes

import concourse.bass as bass
import concourse.mybir as mybir
import concourse.tile as tile
from concourse import bacc
from concourse.bass_utils import run_bass_kernel_spmd
from concourse.masks import make_upper_triangular

# ---- custom DVE exp: exp(x) ~= (1 + x/n + (x/n)^2/2)^n with n = 256 ----
# pass A (EXPA_ANT): u = 1 + s0*x + s1*(s0*x)^2   (s0 = scale/256, s1 = 0.5)
# pass B (EXPB_ANT): u^256 via 8 squarings
# |rel err| ~= x^3/(6*256^2) < 1e-4 for |x|<=3, far below bf16 rounding.
import concourse.dve_ops as _dvo
from concourse.dve_spec import Spec as _Spec, Src0 as _Src0, One as _One, sq as _sq
from concourse.dve_spec import lower as _dve_lower
from concourse.dve_uop import DveOpSpec as _DveOpSpec


def _register_dve_op(name, spec):
    if name in _dvo._SUB_OPCODE_FOR_NAME:
        return next(o for o in _dvo.OPS if o.name == name)
    row = _dvo._CUSTOM_DVE_ROW_BASE + len(_dvo.OPS)
    assert row < 0x20, "custom-DVE opcode rows exhausted"
    _dvo._SUB_OPCODE_FOR_NAME[name] = row
    shas = {
        ver: _DveOpSpec(
            name=name, opcode=row, uops=_dve_lower(spec, ver=ver), rd1_en=False
        ).sha(ver)
        for ver in ("v3", "v4")
    }
    op = _dvo.DveOp(name, spec, subdim=False, uops_sha=shas)
    _dvo.OPS.append(op)
    return op


def _t(s0=None, s1=None):
    from concourse.dve_spec import C0, C1

    t = _Src0 * C0
    return _One + t + _sq(t) * C1


# Offload disabled (measured net loss in 4 schedule variants, incl. a
# hoisted-kb0 form with deferred PV) -> keep the global op registry
# untouched in the grading path. To re-enable:
#   _EXPA = _register_dve_op("EXPA_ANT", _Spec(body=_t(),
#       reference=lambda in0, s0, s1: 1.0 + in0*s0 + np.square(in0*s0)*s1))
#   _EXPB = _register_dve_op("EXPB_ANT", _Spec(
#       body=_sq(_sq(_sq(_sq(_sq(_sq(_sq(_sq(_Src0)))))))),
#       reference=lambda in0: in0.astype(np.float64) ** 256))
_EXPA = _EXPB = None

F32 = mybir.dt.float32
BF16 = mybir.dt.bfloat16
FP8 = mybir.dt.float8e4
DR = mybir.MatmulPerfMode.DoubleRow
EXP = mybir.ActivationFunctionType.Exp
MULT = mybir.AluOpType.mult
ADD = mybir.AluOpType.add

N_CORES = 8
B, S, D = 2, 2048, 1024
N_HEAD = 16
HD = 64  # head dim
HPC = 4  # heads per core
FQK = 2 * HPC * HD  # 512 QK rows
FV = HPC * HD  # 256 V cols
VW = HD + 1  # V block width incl. ones column
NQC = S // 512  # 512-wide q chunks
NKB = S // 128  # 128-wide k blocks
NDC = D // 128  # 128-deep contraction chunks


def build_mha_core(trace_sim=False):
    nc = bacc.Bacc("TRN2", target_bir_lowering=False, debug=False)
    xp_d = nc.dram_tensor("xp", [128, NQC * NDC * 512], BF16, kind="ExternalInput")
    wqkA_d = nc.dram_tensor("wqkA", [128, NDC * 256], BF16, kind="ExternalInput")
    wqkB_d = nc.dram_tensor("wqkB", [128, NDC * 256], BF16, kind="ExternalInput")
    wv_d = nc.dram_tensor("wv", [128, NDC * 256], BF16, kind="ExternalInput")
    qkb_d = nc.dram_tensor("qkb", [128, 4], F32, kind="ExternalInput")
    vb_d = nc.dram_tensor("vb", [128, FV], F32, kind="ExternalInput")
    # 65 rows per head: row 65h = denom/denom junk, rows 65h+1..65h+64 = out
    outT_d = nc.dram_tensor("outT", [HPC * VW, S], F32, kind="ExternalOutput")
    wup_d = nc.dram_tensor("wup", [1, 16], F32, kind="ExternalOutput")

    with tile.TileContext(nc, trace_sim=trace_sim) as tc:
        with (
            tc.tile_pool(name="const", bufs=1) as const,
            tc.tile_pool(name="big", bufs=1) as big,
            tc.tile_pool(name="pp", bufs=4) as pp,
            tc.tile_pool(name="sm", bufs=4) as sm,
            tc.tile_pool(name="ex", bufs=2) as ex,
            tc.tile_pool(name="ps", bufs=3, space="PSUM") as ps,
            tc.tile_pool(name="pav", bufs=1, space="PSUM") as pav,
        ):
            # ---- big SBUF tensors ----
            xsb = big.tile([128, NQC * NDC * 512], BF16)
            wA = big.tile([128, NDC * 256], BF16)
            wB = big.tile([128, NDC * 256], BF16)
            wV = big.tile([128, NDC * 256], BF16)
            qkt = big.tile([128, 4 * S], BF16)  # fc0..3 = Q01,Q23,K01,K23
            vcat = big.tile([128, NKB * HPC * VW], BF16)

            def vcat_view():
                return vcat.rearrange("p (k h j) -> p k h j", k=NKB, h=HPC)
            qkb = const.tile([128, 4], F32)
            vb = const.tile([128, FV], F32)

            # scratch memset first on the gpsimd queue so the PE warmup
            # (which reads it) isn't stuck behind the dma_start instructions
            scr = const.tile([128, 512], BF16)
            nc.gpsimd.memset(scr[:], 0.5)

            # ---- input DMAs (in consumption order; first x chunk split so
            # the first projection matmuls can start sooner) ----
            # weights via the gpsimd DMA queue, x via the sync queue, so
            # both streams start immediately in parallel
            nc.gpsimd.dma_start(out=wA[:, 0:1024], in_=wqkA_d.ap()[:, 0:1024])
            nc.gpsimd.dma_start(out=wA[:, 1024:2048], in_=wqkA_d.ap()[:, 1024:2048])
            nc.gpsimd.dma_start(out=qkb[:], in_=qkb_d.ap())
            nc.gpsimd.dma_start(out=wV[:], in_=wv_d.ap())
            nc.gpsimd.dma_start(out=vb[:], in_=vb_d.ap())
            nc.gpsimd.dma_start(out=wB[:], in_=wqkB_d.ap())
            nc.sync.dma_start(out=xsb[:, 0:1024], in_=xp_d.ap()[:, 0:1024])
            nc.sync.dma_start(out=xsb[:, 1024:2048], in_=xp_d.ap()[:, 1024:2048])
            nc.sync.dma_start(out=xsb[:, 2048:4096], in_=xp_d.ap()[:, 2048:4096])
            for sc in range(1, NQC):
                nc.sync.dma_start(
                    out=xsb[:, sc * 4096 : (sc + 1) * 4096],
                    in_=xp_d.ap()[:, sc * 4096 : (sc + 1) * 4096],
                )

            # ---- constants / warmup (no DMA deps) ----
            mask = const.tile([128, 128], BF16)
            make_upper_triangular(nc, mask[:], val=1.0, diag=True)
            wup_sb = const.tile([1, 16], F32)
            # ACT table preload for Exp happens on first activation: do a
            # tiny one now, during the DMA head.
            nc.scalar.activation(wup_sb[:, 8:16], scr[0:1, 0:8], EXP, scale=1.0)
            # dummy matmuls keep the PE busy ~5us so the HAM clock gate
            # opens before the real projection starts.
            wup_ps = ps.tile([128, 512], F32, tag="ps", name="wup")
            NWUP = 10
            for k in range(NWUP):
                nc.tensor.matmul(
                    wup_ps[:],
                    scr[:, 0:128],
                    scr[:],
                    start=(k == 0),
                    stop=(k == NWUP - 1),
                )
            nc.vector.tensor_copy(out=wup_sb[:, 0:8], in_=wup_ps[0:1, 0:8])

            # ones column of each [1 | V_h] block (denominator rides at
            # partition 0 of av so partition_broadcast reads it directly)
            nc.gpsimd.memset(vcat_view()[:, :, :, 0:1], 1.0)

            def w_slice(fc, dc):
                buf = wA if fc in (0, 2) else wB
                half = 0 if fc in (0, 1) else 128
                return buf[:, dc * 256 + half : dc * 256 + half + 128]

            def emit_qkt(fc, q0, q1):
                """Project Q/K columns [q0, q1) for head-pair column fc.
                q0 must be 512-aligned; q1-q0 is 512 or 1024 (the latter
                spans two x chunks via a strided rhs AP)."""
                n = q1 - q0
                sc = q0 // 512
                pt = ps.tile([128, n], F32, tag="ps", name=f"qk_{fc}_{q0}")
                for dc in range(NDC):
                    if n == 512:
                        rhs = xsb[:, sc * 4096 + dc * 512 : sc * 4096 + dc * 512 + 512]
                    else:
                        rhs = xsb.rearrange("p (sc dc j) -> p sc dc j", sc=NQC, dc=NDC)[
                            :, sc : sc + 2, dc, :
                        ]
                    nc.tensor.matmul(
                        pt[:],
                        w_slice(fc, dc),
                        rhs,
                        start=(dc == 0),
                        stop=(dc == NDC - 1),
                    )
                nc.vector.tensor_scalar_add(
                    qkt[:, fc * S + q0 : fc * S + q1],
                    pt[:],
                    qkb[:, fc : fc + 1],
                )

            def emit_v(kc):
                pt = ps.tile([128, 512], F32, tag="ps", name=f"v_{kc}")
                sc, ko = kc // 4, (kc % 4) * 128
                for dc in range(NDC):
                    nc.tensor.matmul(
                        pt[:, 0:FV],
                        xsb[:, sc * 4096 + dc * 512 + ko : sc * 4096 + dc * 512 + ko + 128],
                        wV[:, dc * 256 : (dc + 1) * 256],
                        start=(dc == 0),
                        stop=(dc == NDC - 1),
                    )
                nc.vector.tensor_tensor(
                    out=vcat_view()[:, kc, :, 1 : HD + 1],
                    in0=pt[:, 0:FV].rearrange("p (h j) -> p h j", h=HPC),
                    in1=vb.rearrange("p (h j) -> p h j", h=HPC),
                    op=ADD,
                )

            def emit_st(pr, qc, kb, st, off):
                qoff = pr * S
                koff = (2 + pr) * S
                for i in (0, 1):
                    nc.tensor.matmul(
                        st[:, i * 512 + off : i * 512 + 512],
                        qkt[64 * i : 64 * i + 64, koff + kb * 128 : koff + kb * 128 + 128],
                        qkt[64 * i : 64 * i + 64, qoff + qc * 512 + off : qoff + qc * 512 + 512],
                        start=True,
                        stop=True,
                    )

            def attn_begin(pr, qc):
                return pav.tile([65, 1024], F32, tag="av", name=f"av_{pr}_{qc}")

            def attn_kbs(pr, qc, av, kbs):
                nkb = 4 * qc + 4
                for kb in kbs:
                    diag = kb // 4 == qc
                    off = 128 * (kb % 4) if diag else 0
                    st = ps.tile([128, 1024], F32, tag="ps", name=f"st_{pr}_{qc}_{kb}")
                    emit_st(pr, qc, kb, st, off)
                    p_t = pp.tile([128, 1024], BF16, tag="p", name=f"p_{pr}_{qc}_{kb}")
                    nc.scalar.activation(
                        p_t.rearrange("p (h q) -> p h q", h=2)[:, :, off:512],
                        st.rearrange("p (h q) -> p h q", h=2)[:, :, off:512],
                        EXP,
                        scale=0.125,
                    )
                    if diag:
                        for i in (0, 1):
                            sl = p_t[:, i * 512 + off : i * 512 + off + 128]
                            nc.vector.tensor_tensor(out=sl, in0=sl, in1=mask[:], op=MULT)
                    for i in (0, 1):
                        h = 2 * pr + i
                        nc.tensor.matmul(
                            av[:, i * 512 + off : i * 512 + 512],
                            vcat_view()[:, kb, h, 0:VW],
                            p_t[:, i * 512 + off : i * 512 + 512],
                            start=(kb == 0),
                            stop=(kb == nkb - 1),
                        )

            def attn_end(pr, qc, av):
                # normalize: out[h] = av[h, 1:65] / av[h, 0]  (denom row 0;
                # row 0 of ou becomes den/den, stripped by the host).
                with tc.high_priority(offset=400):
                    for i in (0, 1):
                        h = 2 * pr + i
                        oc = sm.tile([65, 512], F32, tag="oc", name=f"oc_{pr}_{qc}_{i}")
                        nc.vector.tensor_copy(
                            out=oc[:], in_=av[:, i * 512 : i * 512 + 512]
                        )
                        ss = sm.tile([65, 512], F32, tag="ss", name=f"ss_{pr}_{qc}_{i}")
                        nc.gpsimd.partition_broadcast(ss[:], oc[0:1, :])
                        rr = sm.tile([65, 512], F32, tag="rr", name=f"rr_{pr}_{qc}_{i}")
                        nc.vector.reciprocal_approx_fast(rr[:], ss[:])
                        ou = sm.tile([65, 512], F32, tag="ou", name=f"ou_{pr}_{qc}_{i}")
                        nc.vector.tensor_tensor(out=ou[:], in0=oc[:], in1=rr[:], op=MULT)
                        nc.sync.dma_start(
                            out=outT_d.ap()[VW * h : VW * h + VW, qc * 512 : qc * 512 + 512],
                            in_=ou[:],
                        )

            def emit_attn(pr, qc, fillers=()):
                """Attention for one chunk with projection units (closures)
                interleaved into the kb-loop emission, so the scheduler can
                fill ScalarE-bound stretches with independent matmuls."""
                av = attn_begin(pr, qc)
                nkb = 4 * qc + 4
                fillers = list(fillers)
                nf = len(fillers)
                cut = [(j * nkb) // nf if nf else 0 for j in range(nf)]
                done = 0
                for kb in range(nkb):
                    while done < nf and cut[done] <= kb:
                        fillers[done]()
                        done += 1
                    attn_kbs(pr, qc, av, [kb])
                for f in fillers[done:]:
                    f()
                attn_end(pr, qc, av)

            # ---- pipelined schedule: projection units are interleaved
            # into the attention kb-loops as fillers; pr1 attention is
            # staggered 2 chunks behind pr0 ----
            def QK(fc, sc):
                return lambda: emit_qkt(fc, sc * 512, sc * 512 + 512)

            def V(kc):
                return lambda: emit_v(kc)

            emit_qkt(0, 0, 512)  # Q01 chunk 0
            emit_qkt(2, 0, 512)  # K01 chunk 0
            # w0/w1: no pr1 attention yet -> qkt fillers ride pr0's loop
            # second warmup batch: bridges the early DMA-chasing stretch so
            # the HAM clock gate stays open (a >=3.4us PE idle re-throttles)
            wup2 = ps.tile([128, 512], F32, tag="ps", name="wup2")
            for k in range(10):
                nc.tensor.matmul(
                    wup2[:], scr[:, 0:128], scr[:], start=(k == 0), stop=(k == 9)
                )
            nc.vector.tensor_copy(out=wup_sb[:, 0:8], in_=wup2[0:1, 0:8])
            emit_attn(0, 0, [V(0), V(1), V(2), V(3)])
            emit_qkt(0, 512, 1024)
            emit_qkt(2, 512, 1024)
            emit_qkt(1, 0, 512)
            emit_qkt(3, 0, 512)
            emit_attn(0, 1, [V(4), V(5), V(6), V(7)])
            emit_qkt(0, 1024, 1536)
            emit_qkt(2, 1024, 1536)
            emit_qkt(1, 512, 1024)
            emit_qkt(3, 512, 1024)
            # w2
            emit_attn(0, 2, [V(8), V(9), V(10), V(11)])
            emit_attn(1, 0, [QK(0, 3), QK(2, 3), QK(1, 2), QK(3, 2)])
            # w3
            emit_attn(0, 3, [V(12), V(13), V(14), V(15)])
            emit_attn(1, 1, [QK(1, 3), QK(3, 3)])
            # w4
            emit_attn(1, 2)
            emit_attn(1, 3)
            nc.sync.dma_start(out=wup_d.ap(), in_=wup_sb[:])
    nc.compile()
    return nc


def shard_inputs(x, W_qkv, b_qkv):
    """Full inputs -> list of 8 per-core input maps (host-side packing)."""
    bf = ml_dtypes.bfloat16
    in_maps = []
    for c in range(N_CORES):
        b = c // (N_CORES // B)
        g = c % (N_CORES // B)
        heads = range(HPC * g, HPC * g + HPC)
        qcols = [h * 192 + j for h in heads for j in range(64)]
        kcols = [h * 192 + 64 + j for h in heads for j in range(64)]
        vcols = [h * 192 + 128 + j for h in heads for j in range(64)]

        # x packed [p, sc, dc, j]
        xb = np.asarray(x[b], dtype=np.float32)  # [S, D]
        xpk = (
            xb.reshape(NQC, 512, NDC, 128)
            .transpose(3, 0, 2, 1)
            .reshape(128, NQC * NDC * 512)
        ).astype(bf)

        W = np.asarray(W_qkv, dtype=np.float32)

        def wpack(cols):  # [D, 256] -> [p, dc, 256] -> [128, NDC*256]
            wsh = W[:, cols]  # [1024, 256]
            return (
                wsh.reshape(NDC, 128, 256).transpose(1, 0, 2).reshape(128, NDC * 256)
            ).astype(bf)

        wqkA = wpack(qcols[:128] + kcols[:128])  # Q01 | K01
        wqkB = wpack(qcols[128:] + kcols[128:])  # Q23 | K23
        wv = wpack(vcols)

        b_sh = np.asarray(b_qkv, dtype=np.float32)[qcols + kcols + vcols]
        qkb = np.ascontiguousarray(b_sh[:FQK].reshape(4, 128).T, dtype=np.float32)
        vb = np.ascontiguousarray(
            np.broadcast_to(b_sh[FQK:], (128, FV)), dtype=np.float32
        )
        in_maps.append(
            {"xp": xpk, "wqkA": wqkA, "wqkB": wqkB, "wv": wv, "qkb": qkb, "vb": vb}
        )
    return in_maps


def gather_outputs(results):
    """8 per-core outT [4*65, S] -> full [B, S, D_H] (strip denom rows)."""
    out = np.empty((B, S, N_HEAD * HD), dtype=np.float32)
    for c in range(N_CORES):
        b = c // (N_CORES // B)
        g = c % (N_CORES // B)
        o = results[c]["outT"].reshape(HPC, VW, S)[:, 1:, :]  # [4, 64, S]
        out[b, :, FV * g : FV * (g + 1)] = o.reshape(FV, S).T
    return out


_NC_CACHE = {}


def _get_nc():
    if "nc" not in _NC_CACHE:
        _NC_CACHE["nc"] = build_mha_core()
    return _NC_CACHE["nc"]


def kernel(x, W_qkv, b_qkv, _trace=False, _trace_kwargs=None):
    x = np.asarray(x, dtype=np.float32)
    W_qkv = np.asarray(W_qkv, dtype=np.float32)
    b_qkv = np.asarray(b_qkv, dtype=np.float32)
    nc = _get_nc()
    in_maps = shard_inputs(x, W_qkv, b_qkv)
    res = run_bass_kernel_spmd(
        nc, in_maps, list(range(N_CORES)), trace=_trace, **(_trace_kwargs or {})
    )
    out = gather_outputs(res.results)
    if _trace:
        kernel.last_results = res
    return out

